# revision 31
# baseline (speedup 1.0000x reference)
"""MultiHeadAttention TRN2 kernel.

Math (B=2, H=16, S=2048, D=128, F=256, DIM=2048), all fp32:
  Q = einsum('bhsf,hfd', q, Wq) + bq ; K likewise ; V = einsum('bhse,hed', v, Wv) + bv
  P = softmax(Q K^T / 16) ; o = P V ; out = concat_h(o) @ Wo + bo

Sharding: core c -> batch b=c//4, heads hg=(c%4)*4 .. +4 (tensor parallel over
heads). Each core computes its 4 heads' attention and the partial Wo product
(contraction over its 128*4=512 rows of Wo). Host sums the 4 partials per
batch and adds bo. No device collectives.

Device layout (per core, everything transposed on the host for free):
  qT  [4,2,128,2048] (head j, f-chunk, f, s)   kT same
  vT  [4,128,2048]   (j, e, s)
  wq/wk packed [128, 8*128] (f, (j,fc,d))      wv [128, 4*128] (e, (j,d))
  bq/bk [128,4] (d, j)   bv [128, 4*128] replicated over partitions
  wo [4,128,2048] (j, d, n)
  out_p [2048,2048] = partial (s, n)

All matmuls run as float32r (1 cyc/row at N>=256, full fp32 data).
"""

import os
import sys

import numpy as np

B, H, S, D, F = 2, 16, 2048, 128, 256
DIM = H * D
NC = 8
HPC = 4  # heads per core
SC512 = S // 512  # 4
NKT = S // 128  # 16

_BUILT = None
TRACE = False
LAST_RESULTS = None


def _import_concourse():
    try:
        import concourse.bass  # noqa: F401
    except ImportError:
        sys.path.insert(0, "/opt/trn_rl_repo")


def _build():
    _import_concourse()
    from contextlib import ExitStack

    import concourse.bass as bass
    import concourse.mybir as mybir
    import concourse.tile as tile

    f32 = mybir.dt.float32
    FR = mybir.dt.float32r
    AF = mybir.ActivationFunctionType

    nc = bass.Bass(target_bir_lowering=False)

    qT_d = nc.dram_tensor("qT", [HPC, 2, 128, S], FR, kind="ExternalInput")
    kT_d = nc.dram_tensor("kT", [HPC, 2, 128, S], FR, kind="ExternalInput")
    vT_d = nc.dram_tensor("vT", [HPC, 128, S], FR, kind="ExternalInput")
    wq_d = nc.dram_tensor("wq", [128, HPC * 2 * 128], FR, kind="ExternalInput")
    wk_d = nc.dram_tensor("wk", [128, HPC * 2 * 128], FR, kind="ExternalInput")
    wv_d = nc.dram_tensor("wv", [128, HPC * 128], FR, kind="ExternalInput")
    bq_d = nc.dram_tensor("bq", [128, HPC], f32, kind="ExternalInput")
    bk_d = nc.dram_tensor("bk", [128, HPC], f32, kind="ExternalInput")
    bv_d = nc.dram_tensor("bv", [128, HPC * 128], f32, kind="ExternalInput")
    wo_d = nc.dram_tensor("wo", [HPC, 128, DIM], FR, kind="ExternalInput")
    ones_d = nc.dram_tensor("ones", [128, 128], FR, kind="ExternalInput")
    out_d = nc.dram_tensor("out_p", [S, DIM], f32, kind="ExternalOutput")

    with ExitStack() as ctx:
        tc = ctx.enter_context(tile.TileContext(nc))
        consts = ctx.enter_context(tc.tile_pool(name="consts", bufs=1))
        raw = ctx.enter_context(tc.tile_pool(name="raw", bufs=4))
        big = ctx.enter_context(tc.tile_pool(name="big", bufs=2))
        otn_pool = ctx.enter_context(tc.tile_pool(name="otn", bufs=4))
        sm = ctx.enter_context(tc.tile_pool(name="sm", bufs=2))
        wop = ctx.enter_context(tc.tile_pool(name="wop", bufs=8))
        ps = ctx.enter_context(tc.tile_pool(name="ps", bufs=1, space="PSUM"))

        # ---- constants -------------------------------------------------
        ones_full = consts.tile([128, 128], FR)
        nc.sync.dma_start(out=ones_full, in_=ones_d[:])

        wq_sb = consts.tile([128, HPC * 2 * 128], FR)
        nc.sync.dma_start(out=wq_sb, in_=wq_d[:])
        wk_sb = consts.tile([128, HPC * 2 * 128], FR)
        nc.sync.dma_start(out=wk_sb, in_=wk_d[:])
        wv_sb = consts.tile([128, HPC * 128], FR)
        nc.sync.dma_start(out=wv_sb, in_=wv_d[:])
        bq_sb = consts.tile([128, HPC], f32)
        nc.sync.dma_start(out=bq_sb, in_=bq_d[:])
        bk_sb = consts.tile([128, HPC], f32)
        nc.sync.dma_start(out=bk_sb, in_=bk_d[:])
        bv_sb = consts.tile([128, HPC * 128], f32)
        nc.sync.dma_start(out=bv_sb, in_=bv_d[:])

        otn = []
        for j in range(HPC):
            # ---- P1: load + project head j -----------------------------
            qa = raw.tile([128, S], FR, tag="raw", name=f"qa{j}")
            nc.sync.dma_start(out=qa, in_=qT_d[j, 0])
            qb = raw.tile([128, S], FR, tag="raw", name=f"qb{j}")
            nc.sync.dma_start(out=qb, in_=qT_d[j, 1])
            ka = raw.tile([128, S], FR, tag="raw", name=f"ka{j}")
            nc.sync.dma_start(out=ka, in_=kT_d[j, 0])
            kb = raw.tile([128, S], FR, tag="raw", name=f"kb{j}")
            nc.sync.dma_start(out=kb, in_=kT_d[j, 1])
            va = raw.tile([128, S], FR, tag="raw", name=f"va{j}")
            nc.gpsimd.dma_start(out=va, in_=vT_d[j])

            QT = big.tile([128, S], FR, tag="QT", name=f"QT{j}")
            KT = big.tile([128, S], FR, tag="KT", name=f"KT{j}")
            Vsb = big.tile([128, S], FR, tag="V", name=f"V{j}")

            for sc in range(SC512):
                ssl = slice(sc * 512, (sc + 1) * 512)
                pq = ps.tile([128, 512], f32, tag="w", bufs=2, name=f"pq{j}_{sc}")
                nc.tensor.matmul(pq, (wq_sb[:, (j * 2 + 0) * 128 : (j * 2 + 1) * 128]),
                                 qa[:, ssl], start=True, stop=False)
                nc.tensor.matmul(pq, (wq_sb[:, (j * 2 + 1) * 128 : (j * 2 + 2) * 128]),
                                 qb[:, ssl], start=False, stop=True)
                nc.vector.tensor_scalar_add(out=QT[:, ssl], in0=pq, scalar1=bq_sb[:, j : j + 1])
                pk = ps.tile([128, 512], f32, tag="w", bufs=2, name=f"pk{j}_{sc}")
                nc.tensor.matmul(pk, (wk_sb[:, (j * 2 + 0) * 128 : (j * 2 + 1) * 128]),
                                 ka[:, ssl], start=True, stop=False)
                nc.tensor.matmul(pk, (wk_sb[:, (j * 2 + 1) * 128 : (j * 2 + 2) * 128]),
                                 kb[:, ssl], start=False, stop=True)
                nc.vector.tensor_scalar_add(out=KT[:, ssl], in0=pk, scalar1=bk_sb[:, j : j + 1])

            for kt in range(NKT):
                csl = slice(kt * 128, (kt + 1) * 128)
                pv = ps.tile([128, 512], f32, tag="w", bufs=2, name=f"pv{j}_{kt}")
                nc.tensor.matmul(pv[:, 0:128], va[:, csl],
                                 wv_sb[:, j * 128 : (j + 1) * 128], start=True, stop=True)
                nc.vector.tensor_add(out=Vsb[:, csl], in0=pv[:, 0:128],
                                     in1=bv_sb[:, j * 128 : (j + 1) * 128])

            # ---- P2: attention head j ----------------------------------
            oTn = otn_pool.tile([128, S], FR, tag="otn", name=f"oTn{j}")
            otn.append(oTn)
            for qc in range(SC512):
                qsl = slice(qc * 512, (qc + 1) * 512)
                po = ps.tile([128, 512], f32, tag="o", bufs=2, name=f"po{j}_{qc}")
                pr = ps.tile([128, 512], f32, tag="r", bufs=1, name=f"pr{j}_{qc}")

                def emit_pscore(kt):
                    csl = slice(kt * 128, (kt + 1) * 128)
                    t = ps.tile([128, 512], f32, tag="s", bufs=3,
                                name=f"ps{j}_{qc}_{kt}")
                    nc.tensor.matmul(t, KT[:, csl], QT[:, qsl],
                                     start=True, stop=True)
                    return t

                # software pipeline: pscore(kt+1) is emitted before po(kt)
                # so PE's in-order queue keeps ACT fed with score tiles
                # while po waits on exp(kt); otherwise every exp gets a
                # PE->ACT round-trip bubble on the bottleneck engine
                cur = emit_pscore(0)
                for kt in range(NKT):
                    csl = slice(kt * 128, (kt + 1) * 128)
                    pT = sm.tile([128, 512], FR, tag="pT", bufs=3, name=f"pT{j}_{qc}_{kt}")
                    nc.scalar.activation(out=pT, in_=cur, func=AF.Exp,
                                         bias=0.0, scale=0.0625)
                    if kt + 1 < NKT:
                        cur = emit_pscore(kt + 1)
                    nc.tensor.matmul(po, Vsb[:, csl], pT,
                                     start=(kt == 0), stop=(kt == NKT - 1))
                    nc.tensor.matmul(pr, ones_full, pT,
                                     start=(kt == 0), stop=(kt == NKT - 1))
                rr = sm.tile([128, 512], f32, tag="rr_sb", bufs=2, name=f"rr{j}_{qc}")
                nc.vector.reciprocal(out=rr, in_=pr)
                nc.vector.tensor_mul(out=oTn[:, qsl], in0=po, in1=rr)

        # ---- P3: out_p = sum_j oTn_j.T @ wo_j --------------------------
        for dc in range(DIM // 512):
            dsl = slice(dc * 512, (dc + 1) * 512)
            wo_sb = []
            for j in range(HPC):
                w = wop.tile([128, 512], FR, tag="wo", name=f"wo{dc}_{j}")
                nc.scalar.dma_start(out=w, in_=wo_d[j, :, dsl])
                wo_sb.append(w)
            for sc in range(S // 128):
                csl = slice(sc * 128, (sc + 1) * 128)
                pw = ps.tile([128, 512], f32, tag="w", bufs=2, name=f"pw{dc}_{sc}")
                for j in range(HPC):
                    nc.tensor.matmul(pw, otn[j][:, csl], wo_sb[j],
                                     start=(j == 0), stop=(j == HPC - 1))
                ow = sm.tile([128, 512], f32, tag="ow", bufs=3, name=f"ow{dc}_{sc}")
                nc.scalar.copy(out=ow, in_=pw)
                eng = nc.gpsimd if (sc % 2 == 0) else nc.sync
                eng.dma_start(out=out_d[csl, dsl], in_=ow)

    _split_excess_waits(nc)
    return nc


def _split_excess_waits(nc):
    """Compute-engine instructions (Matmult, TensorScalarPtr, ...) only have
    one sync-wait slot in walrus codegen. Split any excess waits onto
    same-engine NoOps inserted just before the instruction."""
    import concourse.mybir as mybir

    n = 0
    for func in nc.m.functions:
        for block in func.blocks:
            out = []
            for inst in block.instructions:
                si = getattr(inst, "sync_info", None)
                if si is not None and si.on_wait and len(si.on_wait) > 1:
                    for w in si.on_wait[:-1]:
                        nop = mybir.InstNoOp(
                            name=f"wsplit_{n}",
                            engine=inst.engine,
                            sync_info=mybir.SyncInfo(on_wait=[w], on_update=[]),
                            bass_nofuse=True,
                        )
                        n += 1
                        out.append(nop)
                    inst.sync_info = mybir.SyncInfo(
                        on_wait=[si.on_wait[-1]], on_update=si.on_update)
                out.append(inst)
            block.instructions[:] = out
    return n


def _prep_core(c, q, k, v, Wq, Wk, Wv, bq, bk, bv, Wo):
    b = c // 4
    hs = slice((c % 4) * HPC, (c % 4) * HPC + HPC)
    qT = np.ascontiguousarray(q[b, hs].transpose(0, 2, 1)).reshape(HPC, 2, 128, S)
    kT = np.ascontiguousarray(k[b, hs].transpose(0, 2, 1)).reshape(HPC, 2, 128, S)
    vT = np.ascontiguousarray(v[b, hs].transpose(0, 2, 1))
    wq = np.ascontiguousarray(
        Wq[hs].reshape(HPC, 2, 128, D).transpose(2, 0, 1, 3)).reshape(128, HPC * 2 * 128)
    wk = np.ascontiguousarray(
        Wk[hs].reshape(HPC, 2, 128, D).transpose(2, 0, 1, 3)).reshape(128, HPC * 2 * 128)
    wv = np.ascontiguousarray(Wv[hs].transpose(1, 0, 2)).reshape(128, HPC * 128)
    bqT = np.ascontiguousarray(bq[hs].T)
    bkT = np.ascontiguousarray(bk[hs].T)
    bvr = np.ascontiguousarray(
        np.broadcast_to(bv[hs][:, None, :], (HPC, 128, D)).transpose(1, 0, 2)
    ).reshape(128, HPC * D)
    wo = np.ascontiguousarray(Wo.reshape(H, D, DIM)[hs])
    return {
        "qT": qT, "kT": kT, "vT": vT, "wq": wq, "wk": wk, "wv": wv,
        "bq": bqT, "bk": bkT, "bv": bvr, "wo": wo,
        "ones": np.ones((128, 128), dtype=np.float32),
    }


def kernel(q, k, v, Wq, Wk, Wv, bq, bk, bv, Wo, bo):
    global _BUILT, LAST_RESULTS
    _import_concourse()
    from concourse.bass_utils import run_bass_kernel_spmd

    args = [np.asarray(x, dtype=np.float32)
            for x in (q, k, v, Wq, Wk, Wv, bq, bk, bv, Wo)]
    if _BUILT is None:
        _BUILT = _build()
    in_maps = [_prep_core(c, *args) for c in range(NC)]
    res = run_bass_kernel_spmd(_BUILT, in_maps, core_ids=list(range(NC)),
                               trace=TRACE)
    LAST_RESULTS = res
    bo = np.asarray(bo, dtype=np.float32)
    outs = [res.results[c]["out_p"] for c in range(NC)]
    out = np.stack([
        outs[0] + outs[1] + outs[2] + outs[3] + bo,
        outs[4] + outs[5] + outs[6] + outs[7] + bo,
    ]).astype(np.float32)
    return out
